# revision 1
# baseline (speedup 1.0000x reference)
"""Multi-head attention (B=2, S=2048, D=1024, H=16, Dh=64) on 8 TRN2 cores.

Sharding: data-parallel over batch (2) x tensor-parallel over heads (16 -> 4
groups of 4). Core c handles batch c//4, heads [4*(c%4), 4*(c%4)+4).
Each core computes its partial output projection (Wo column slice); the host
sums the 4 partials per batch (the "all-reduce") and adds bo.

Device-side per core:
  Q_T/K_T/V_T = W @ X.T via PE, V transposed back to [s, j] via PE transpose.
  Per head: scores_T[k,q] = (K_T-tile).T @ Q_T (K=64 contraction, psum f32),
  exp on ACT (no max subtraction: scores ~ N(0,1), exp never overflows f32),
  attn@V with a ones-column appended to V so row 64 of the PSUM accumulator
  collects the softmax denominator; normalize via DVE reciprocal + a K=1
  ones-matmul partition-broadcast; then the Wo projection in fp32r.

Schedule choices (driven by the cost model; PE executes in emission order):
  - q/k stream + project first so the exp pipeline starts as early as
    possible; Q_T/K_T psum evacuation on ScalarE (idle until the first exp)
  - the V projection/transpose jobs are interleaved INTO the first heads'
    attention loops (xv arrives during attention; the attn@V lag behind exp
    is absorbed by a deep exp-tile pool); V jobs borrow the spare attn@V
    accumulator PSUM slot so scores keep double-buffering
  - normalization of head-task i is emitted after head-task i+1's kb-loop
    so the PE never stalls on the DVE reciprocal
  - attention runs q-half-outer; the first q-half's output projection is
    emitted in slices interleaved into the second q-half's attention

ATT_F16=True streams x/w and runs the attention matmuls in fp16 (~1e-3
rel err, halves the DMA); False keeps everything fp32r (~4e-4 rel err).
Scores, softmax denominators and all PSUM accumulation stay f32 either way.
"""

import numpy as np
from contextlib import ExitStack

import concourse.bass as bass
from concourse import bacc
import concourse.mybir as mybir
import concourse.tile as tile

F32 = mybir.dt.float32
F32R = mybir.dt.float32r
F16 = mybir.dt.float16
AF = mybir.ActivationFunctionType

ATT_F16 = True

B = 2
S = 2048
D = 1024
H = 16
DH = 64
NCORES = 8
HL = 4          # heads per core
J = HL * DH     # 256 local projection width
P = 128
KD = D // P     # 8 d-chunks
NS = S // 512   # 4 s-tiles of 512
KB = S // P     # 16 k-blocks
QH = S // 1024  # 2 q-halves of 1024
EB = D // P     # 8 e-blocks

XDT = F16 if ATT_F16 else F32R        # streamed x / w dtype for q,k,v path
EX_BUFS = 10 if ATT_F16 else 8


def build_nc():
    nc = bacc.Bacc()

    xq = nc.dram_tensor("xq", [P, KD, S], XDT, kind="ExternalInput")
    xk = nc.dram_tensor("xk", [P, KD, S], XDT, kind="ExternalInput")
    xv = nc.dram_tensor("xv", [P, KD, S], XDT, kind="ExternalInput")
    wq = nc.dram_tensor("wq", [P, KD, J], XDT, kind="ExternalInput")
    wk = nc.dram_tensor("wk", [P, KD, J], XDT, kind="ExternalInput")
    wv = nc.dram_tensor("wv", [P, KD, J], XDT, kind="ExternalInput")
    wo = nc.dram_tensor("wo", [P, 2, D], F32R, kind="ExternalInput")
    out_t = nc.dram_tensor("out_t", [EB, P, S], F32, kind="ExternalOutput")

    with tile.TileContext(nc) as tc, ExitStack() as st:
        const = st.enter_context(tc.tile_pool(name="const", bufs=1))
        persist = st.enter_context(tc.tile_pool(name="persist", bufs=1))
        xpool = st.enter_context(tc.tile_pool(name="xstream", bufs=8 if ATT_F16 else 4))

        wq_sb = const.tile([P, KD, J], XDT, tag="wq")
        wk_sb = const.tile([P, KD, J], XDT, tag="wk")
        wv_sb = const.tile([P, KD, J], XDT, tag="wv")
        wo_sb = const.tile([P, 2, D], F32R, tag="wo")

        qt_sb = persist.tile([P, 2, S], XDT, tag="qt")   # Q_T [256, 2048]
        kt_sb = persist.tile([P, 2, S], XDT, tag="kt")   # K_T
        vt_sb = persist.tile([P, 2, S], XDT, tag="vt")   # V_T, pre-transpose
        v_sb = persist.tile([P, KB, HL * (DH + 1)], XDT, tag="v")  # V + ones
        ao_sb = persist.tile([P, 2, S], F32R, tag="ao")  # normalized attn out ^T

        identity = const.tile([P, P], XDT, tag="ident")
        ones64 = const.tile([1, DH], F32R, tag="ones64")

        # wv + xv stream first (V projection overlaps its own DMA and the
        # q/k stream); q/k weights are emitted after the xv chunks below
        nc.sync.dma_start(out=wv_sb[:], in_=wv[:])

        from concourse.masks import make_identity
        if ATT_F16:
            make_identity(nc, identity[:])
            ones_dram = nc.inline_tensor(np.ones((P, KB), np.float16), name="ones_c")
            ones_ap = ones_dram.ap()
        else:
            make_identity(nc, identity[:].bitcast(F32))
            ones_dram = nc.inline_tensor(np.ones((P, KB), np.float32), name="ones_c")
            ones_ap = ones_dram.ap().bitcast(F32R)

        # --- Q/K projections (dc-outer over 8 psum accumulators each) ---
        def projection(src, wsb, dst, pproj, evac_dve=False):
            psums = [
                pproj.tile([P, 512], F32, tag="pp", name=f"pp{i}")
                for i in range(2 * NS)
            ]
            for dc in range(KD):
                xc = xpool.tile([P, S], XDT, tag="xc", name=f"xc{dc}")
                nc.sync.dma_start(out=xc[:], in_=src[:, dc, :])
                for jb in range(2):
                    for stl in range(NS):
                        nc.tensor.matmul(
                            psums[jb * NS + stl][:],
                            wsb[:, dc, jb * P:(jb + 1) * P],
                            xc[:, stl * 512:(stl + 1) * 512],
                            start=(dc == 0),
                            stop=(dc == KD - 1),
                        )
            # evacuation (jb0 first); ScalarE is idle before the first exp
            for jb in range(2):
                for stl in range(NS):
                    d = dst[:, jb, stl * 512:(stl + 1) * 512]
                    if evac_dve:
                        nc.vector.tensor_copy(d, psums[jb * NS + stl][:])
                    else:
                        nc.scalar.copy(d, psums[jb * NS + stl][:])

        with tc.tile_pool(name="pproj", bufs=8, space="PSUM") as pproj:
            projection(xv, wv_sb, vt_sb, pproj, evac_dve=True)
            nc.sync.dma_start(out=wq_sb[:], in_=wq[:])
            nc.sync.dma_start(out=wk_sb[:], in_=wk[:])
            projection(xq, wq_sb, qt_sb, pproj)
            projection(xk, wk_sb, kt_sb, pproj)
        nc.sync.dma_start(out=wo_sb[:], in_=wo[:])
        for h in range(HL):
            nc.sync.dma_start(out=v_sb[:, :, h * (DH + 1) + DH], in_=ones_ap)
        ones_f32 = nc.inline_tensor(np.ones((1, DH), np.float32), name="ones_f")
        nc.sync.dma_start(out=ones64[:], in_=ones_f32.ap().bitcast(F32R))

        # --- attention + deferred V pipeline + interleaved output proj ---
        with tc.tile_pool(name="psc", bufs=2, space="PSUM") as psc, tc.tile_pool(
            name="poacc", bufs=2, space="PSUM"
        ) as poacc, tc.tile_pool(name="expp", bufs=EX_BUFS) as expp, tc.tile_pool(
            name="npool", bufs=2
        ) as npool, tc.tile_pool(name="ostage", bufs=4) as opool:

            # V transposes, emitted lazily inside the first heads' kb-loops
            # (vt_sb is ready before attention starts; these fill PE slack
            # and borrow the spare "oacc" PSUM slot)
            def vjob_transpose(sb, jb):
                def f():
                    tp = poacc.tile([P, P], XDT, tag="oacc",
                                    name=f"tp_{sb}_{jb}")
                    nc.tensor.transpose(
                        tp[:, :P], vt_sb[:, jb, sb * P:(sb + 1) * P], identity[:]
                    )
                    for hh in range(2):
                        h = jb * 2 + hh
                        nc.vector.tensor_copy(
                            v_sb[:, sb, h * (DH + 1):h * (DH + 1) + DH],
                            tp[:, hh * DH:(hh + 1) * DH],
                        )
                return f

            vjobs = []
            for sb in range(KB):
                vjobs.append(vjob_transpose(sb, 0))
                vjobs.append(vjob_transpose(sb, 1))

            def kb_loop(qh, h, vjob_budget=0, mid_cb=None, norm_cb=None):
                q0 = qh * 1024
                jb = h // 2
                off = DH * (h % 2)
                oacc = poacc.tile([DH + 1, 1024], F32, tag="oacc")
                for kb in range(KB):
                    sc = psc.tile([P, 1024], F32, tag="sc")
                    for n in range(2):
                        nc.tensor.matmul(
                            sc[:, n * 512:(n + 1) * 512],
                            kt_sb[off:off + DH, jb, kb * P:(kb + 1) * P],
                            qt_sb[off:off + DH, jb, q0 + n * 512:q0 + (n + 1) * 512],
                            start=True,
                            stop=True,
                        )
                    ex = expp.tile([P, 1024], XDT, tag="ex")
                    nc.scalar.activation(ex[:], sc[:], AF.Exp)
                    for _ in range(vjob_budget):
                        if vjobs:
                            vjobs.pop(0)()
                    if norm_cb is not None and kb == KB // 4:
                        norm_cb()
                    if mid_cb is not None and kb == KB // 2:
                        mid_cb()
                    for n in range(2):
                        nc.tensor.matmul(
                            oacc[:, n * 512:(n + 1) * 512],
                            v_sb[:, kb, h * (DH + 1):(h + 1) * (DH + 1)],
                            ex[:, n * 512:(n + 1) * 512],
                            start=(kb == 0),
                            stop=(kb == KB - 1),
                        )
                recip = npool.tile([1, 1024], F32R, tag="recip")
                with nc.allow_low_precision(reason="fp32r softmax denom"):
                    nc.vector.reciprocal(recip[:], oacc[DH:DH + 1, :])
                return oacc, recip

            def normalize(task_state):
                (qh, h), (oacc, recip) = task_state
                q0 = qh * 1024
                jb = h // 2
                off = DH * (h % 2)
                bc = psc.tile([DH, 1024], F32, tag="sc")
                for n in range(2):
                    nc.tensor.matmul(
                        bc[:, n * 512:(n + 1) * 512],
                        ones64[:],
                        recip[:, n * 512:(n + 1) * 512],
                        start=True,
                        stop=True,
                    )
                bcast = npool.tile([DH, 1024], F32, tag="bcast")
                nc.vector.tensor_copy(bcast[:], bc[:])
                nc.vector.tensor_mul(
                    ao_sb[off:off + DH, jb, q0:q0 + 1024],
                    oacc[0:DH, :],
                    bcast[:],
                )

            def oproj_slice(qh, ebs):
                q0 = qh * 1024
                for eb in ebs:
                    ob = opool.tile([P, 1024], F32, tag="ob")
                    for stl in range(2):
                        s0 = q0 + stl * 512
                        po = poacc.tile([P, 512], F32, tag="oacc",
                                        name=f"po_{qh}_{eb}_{stl}")
                        for jb in range(2):
                            nc.tensor.matmul(
                                po[:, :512],
                                wo_sb[:, jb, eb * P:(eb + 1) * P],
                                ao_sb[:, jb, s0:s0 + 512],
                                start=(jb == 0),
                                stop=(jb == 1),
                            )
                        d = ob[:, stl * 512:(stl + 1) * 512]
                        if qh == 1 and stl == 0:
                            nc.scalar.copy(d, po[:, :512])  # ACT idle at tail
                        else:
                            nc.vector.tensor_copy(d, po[:, :512])
                    nc.sync.dma_start(out=out_t[eb][:, q0:q0 + 1024], in_=ob[:])

            tasks = [(qh, h) for qh in range(QH) for h in range(HL)]
            pending = [None]
            for i, (qh, h) in enumerate(tasks):
                # sprinkle V transposes into the first task's PE slack;
                # the previous task's normalize lands at kb=4 (frees its
                # accumulator slot); qh0's output projection creeps through
                # qh1 one eb at a time (kb=8 + task end)
                def norm_prev():
                    if pending[0] is not None:
                        normalize(pending[0])
                        pending[0] = None
                mid = None
                if 4 <= i <= 7:
                    eb_mid = (i - 4) * 2
                    mid = lambda e=eb_mid: oproj_slice(0, [e])
                state = kb_loop(qh, h, vjob_budget=4 if i < 1 else 0,
                                mid_cb=mid, norm_cb=norm_prev)
                assert not vjobs or i < 1
                pending[0] = ((qh, h), state)
                if 4 <= i <= 7:
                    oproj_slice(0, [(i - 4) * 2 + 1])
            normalize(pending[0][1] and pending[0])
            oproj_slice(1, range(EB))

    nc.finalize()
    return nc


_NC_CACHE = None


def _get_nc():
    global _NC_CACHE
    if _NC_CACHE is None:
        _NC_CACHE = build_nc()
    return _NC_CACHE


def make_in_maps(query, key, value, Wq, Wk, Wv, Wo):
    """Build the 8 per-core input dicts from the full tensors (p-major)."""
    query = np.asarray(query, np.float32)
    key = np.asarray(key, np.float32)
    value = np.asarray(value, np.float32)
    Wq = np.asarray(Wq, np.float32)
    Wk = np.asarray(Wk, np.float32)
    Wv = np.asarray(Wv, np.float32)
    Wo = np.asarray(Wo, np.float32)
    xdt = np.float16 if ATT_F16 else np.float32

    def pmajor(a2d, inner):  # [Drows, inner] -> [P, Drows//P, inner]
        return np.ascontiguousarray(
            a2d.reshape(KD, P, inner).transpose(1, 0, 2)
        )

    scale = np.float32(1.0 / np.sqrt(DH))
    xs = {}
    for b in range(B):
        xs[b] = {
            "xq": pmajor(np.ascontiguousarray(query[b].T), S).astype(xdt),
            "xk": pmajor(np.ascontiguousarray(key[b].T), S).astype(xdt),
            "xv": pmajor(np.ascontiguousarray(value[b].T), S).astype(xdt),
        }
    ws = {}
    for hg in range(4):
        sl = slice(hg * J, (hg + 1) * J)
        wo_t = np.ascontiguousarray(Wo[:, sl].T)  # [256, 1024]
        ws[hg] = {
            "wq": pmajor(np.ascontiguousarray(Wq[sl].T * scale), J).astype(xdt),
            "wk": pmajor(np.ascontiguousarray(Wk[sl].T), J).astype(xdt),
            "wv": pmajor(np.ascontiguousarray(Wv[sl].T), J).astype(xdt),
            "wo": np.ascontiguousarray(
                wo_t.reshape(2, P, D).transpose(1, 0, 2)
            ),
        }
    in_maps = []
    for c in range(NCORES):
        b, hg = c // 4, c % 4
        m = {}
        m.update(xs[b])
        m.update(ws[hg])
        in_maps.append(m)
    return in_maps


def assemble(results, bo):
    """Sum the 4 per-core partials per batch, add bo."""
    bo = np.asarray(bo, np.float32)
    out = np.zeros((B, S, D), np.float32)
    for c in range(NCORES):
        b = c // 4
        part = results[c]["out_t"].reshape(D, S).T  # [S, D]
        out[b] += part
    out += bo[None, None, :]
    return out


def kernel(query, key, value, Wq, Wk, Wv, Wo, bo):
    import os
    import time

    # helps recover wedged NeuronCores between runs
    os.environ.setdefault("NEURON_RT_RESET_CORES", "1")
    from concourse.bass_utils import run_bass_kernel_spmd

    nc = _get_nc()
    in_maps = make_in_maps(query, key, value, Wq, Wk, Wv, Wo)
    last_exc = None
    for attempt in range(3):
        try:
            res = run_bass_kernel_spmd(nc, in_maps, list(range(NCORES)))
            return assemble(res.results, bo)
        except Exception as e:  # transient NRT_EXEC_UNIT_UNRECOVERABLE etc.
            last_exc = e
            time.sleep(2.0)
    raise last_exc



# revision 15
# speedup vs baseline: 1.0882x; 1.0882x over previous
"""Multi-head attention (B=2, S=2048, D=1024, H=16, Dh=64) on 8 TRN2 cores.

Sharding: data-parallel over batch (2) x tensor-parallel over heads (16 -> 4
groups of 4). Core c handles batch c//4, heads [4*(c%4), 4*(c%4)+4).
Each core computes its partial output projection (Wo column slice); the host
sums the 4 per-core partials per batch (the "all-reduce") and adds bo.

All-fp16 data path (fp8/DoubleRow was tried and is numerically dead here:
quantization noise on scores/weights/values does NOT average down — the
attention output is a weighted mean whose magnitude shrinks as fast as the
noise, so fp8 anywhere in the value path lands at ~3-7% output error vs the
2e-2 gate; fp16 gives ~6e-4).

The kernel is PE-bound (fp16 matmul floor ~401k PE cycles = 167us at the
2.4GHz max p-state vs ACT's 133us exp stream), and the TRN2 p-state model
punishes PE idle gaps (any gap resets the clock ramp). Schedule:

  - V is projected directly in natural [s, j] layout (x chunk stationary,
    Wv moving): no PE transposes of V^T.
  - Head: Q/K jb0 halves project dc-outer (paced by x DMA on two HWDGE
    queues); jb1 halves + V s-blocks interleave into the first tasks.
  - Main loop: per kb, scores + exp + lag-1 attn@V; filler jobs keep the
    PE dense under the ACT exp stream.
  - qh0's output projection interleaves into tasks 4-7; qh1's runs at the
    tail with evacuation alternating DVE/ACT. Out streams per-eb as fp16.
"""

import numpy as np
from collections import deque
from contextlib import ExitStack

import concourse.bass as bass
from concourse import bacc
import concourse.mybir as mybir
import concourse.tile as tile

F32 = mybir.dt.float32
F32R = mybir.dt.float32r
F16 = mybir.dt.float16
AF = mybir.ActivationFunctionType

B = 2
S = 2048
D = 1024
H = 16
DH = 64
NCORES = 8
HL = 4          # heads per core
J = HL * DH     # 256 local projection width
P = 128
KD = D // P     # 8 d-chunks
KB = S // P     # 16 k-blocks
QH = S // 1024  # 2 q-halves of 1024
EB = D // P     # 8 e-blocks


def build_nc():
    nc = bacc.Bacc()

    NS = 4  # s-chunks of 512
    xq = nc.dram_tensor("xq", [P, NS, KD, 512], F16, kind="ExternalInput")
    xk = nc.dram_tensor("xk", [P, NS, KD, 512], F16, kind="ExternalInput")
    xv = nc.dram_tensor("xv", [P, NS, KD, 512], F16, kind="ExternalInput")
    wq = nc.dram_tensor("wq", [P, KD, J], F16, kind="ExternalInput")
    wk = nc.dram_tensor("wk", [P, KD, J], F16, kind="ExternalInput")
    wv = nc.dram_tensor("wv", [P, KD, J], F16, kind="ExternalInput")
    wo = nc.dram_tensor("wo", [P, 2, D], F16, kind="ExternalInput")
    out_t = nc.dram_tensor("out_t", [EB, P, S], F16, kind="ExternalOutput")

    with tile.TileContext(nc) as tc, ExitStack() as st:
        const = st.enter_context(tc.tile_pool(name="const", bufs=1))
        persist = st.enter_context(tc.tile_pool(name="persist", bufs=1))
        xpool = st.enter_context(tc.tile_pool(name="xstream", bufs=12))

        wq_sb = const.tile([P, KD, J], F16, tag="wq")
        wk_sb = const.tile([P, KD, J], F16, tag="wk")
        wv_sb = const.tile([P, KD, J], F16, tag="wv")
        wo_sb = const.tile([P, 2, D], F16, tag="wo")

        qt_sb = persist.tile([P, 2, S], F16, tag="qt")   # Q_T [256, 2048]
        kt_sb = persist.tile([P, 2, S], F16, tag="kt")   # K_T
        # V natural layout + per-head ones col: [s_part, kb, h, 65]
        v_sb = persist.tile([P, KB, HL, DH + 1], F16, tag="v")
        ao_sb = persist.tile([P, 2, S], F16, tag="ao")  # normalized attn out ^T

        ones64 = const.tile([1, DH], F32R, tag="ones64")

        # --- DMA: one serial stream; order = consumption order -----------
        xq_t = [xpool.tile([P, KD, 512], F16, tag="xc", name=f"xq{c}")
                for c in range(NS)]
        xk_t = [xpool.tile([P, KD, 512], F16, tag="xc", name=f"xk{c}")
                for c in range(NS)]
        xv_t = [xpool.tile([P, KD, 512], F16, tag="xc", name=f"xv{c}")
                for c in range(NS)]
        ones16 = nc.inline_tensor(np.ones((P, KB * HL), np.float16),
                                  name="ones16")
        ones_f32 = nc.inline_tensor(np.ones((1, DH), np.float32), name="ones_f")
        dma_order = [
            (wq_sb, wq, None), (wk_sb, wk, None),
            (xk_t[0], xk, 0), (xq_t[0], xq, 0), (xq_t[1], xq, 1),
            (xk_t[1], xk, 1), (xk_t[2], xk, 2), (xk_t[3], xk, 3),
            (wv_sb, wv, None),
            (xv_t[0], xv, 0), (xq_t[2], xq, 2), (xv_t[1], xv, 1),
            (xq_t[3], xq, 3), (xv_t[2], xv, 2), (xv_t[3], xv, 3),
        ]
        for dst, srcd, sch in dma_order:
            if sch is None:
                nc.sync.dma_start(out=dst[:], in_=srcd[:])
            else:
                nc.sync.dma_start(out=dst[:], in_=srcd[:, sch])
        nc.sync.dma_start(out=v_sb[:, :, :, DH], in_=ones16.ap())
        nc.sync.dma_start(out=ones64[:], in_=ones_f32.ap().bitcast(F32R))
        nc.sync.dma_start(out=wo_sb[:], in_=wo[:])

        # --- attention pipeline with interleaved filler work --------------
        with tc.tile_pool(name="psc", bufs=2, space="PSUM") as psc, tc.tile_pool(
            name="poacc", bufs=2, space="PSUM"
        ) as poacc, tc.tile_pool(name="expp", bufs=17) as expp, tc.tile_pool(
            name="npool", bufs=2
        ) as npool, tc.tile_pool(name="ostage", bufs=4) as opool:

            def proj_job(wsb, xts, dst, sch, jb, nm):
                def f():
                    pp = psc.tile([P, 512], F32, tag="sc",
                                  name=f"pj{nm}{sch}{jb}")
                    for c in range(KD):
                        nc.tensor.matmul(
                            pp[:, :512],
                            wsb[:, c, jb * P:(jb + 1) * P],
                            xts[sch][:, c, :],
                            start=(c == 0),
                            stop=(c == KD - 1),
                        )
                    nc.vector.tensor_copy(
                        dst[:, jb, sch * 512:(sch + 1) * 512], pp[:])
                return f

            def vjob(sb):
                def f():
                    vp = psc.tile([P, J], F32, tag="sc", name=f"vp{sb}")
                    for c in range(KD):
                        nc.tensor.matmul(
                            vp[:, :J],
                            xv_t[sb // 4][:, c, (sb % 4) * P:(sb % 4 + 1) * P],
                            wv_sb[:, c, :],
                            start=(c == 0),
                            stop=(c == KD - 1),
                        )
                    nc.vector.tensor_copy(v_sb[:, sb, :, 0:DH], vp[:])
                return f

            def oproj_eb(qh, eb, evac="dve", tailpool=False):
                def f():
                    q0 = qh * 1024
                    ob = opool.tile([P, 1024], F16, tag="ob",
                                    name=f"ob{(qh * EB + eb) % 4}")
                    for stl in range(2):
                        s0 = q0 + stl * 512
                        pool = poacc if tailpool else psc
                        tg = "oacc" if tailpool else "sc"
                        po = pool.tile([P, 512], F32, tag=tg,
                                       name=f"po_{qh}_{eb}_{stl}")
                        for jbx in range(2):
                            nc.tensor.matmul(
                                po[:, :512],
                                wo_sb[:, jbx, eb * P:(eb + 1) * P],
                                ao_sb[:, jbx, s0:s0 + 512],
                                start=(jbx == 0),
                                stop=(jbx == 1),
                            )
                        d = ob[:, stl * 512:(stl + 1) * 512]
                        if evac == "act" or (evac == "mix" and stl == 0):
                            nc.scalar.copy(d, po[:, :512])
                        else:
                            nc.vector.tensor_copy(d, po[:, :512])
                    nc.sync.dma_start(out=out_t[eb][:, q0:q0 + 1024],
                                      in_=ob[:])
                return f

            # filler placement (task, kb) tuned against the cost model;
            # K jb0 s-chunks are DMA-deadline-paced for task0's kb sweep
            fillers = {}

            def put(i, kb, fn):
                fillers.setdefault((i, kb), []).append(fn)

            put(0, 3, proj_job(wk_sb, xk_t, kt_sb, 1, 0, "k"))
            put(0, 7, proj_job(wk_sb, xk_t, kt_sb, 2, 0, "k"))
            put(0, 10, proj_job(wk_sb, xk_t, kt_sb, 3, 0, "k"))
            put(0, 0, proj_job(wk_sb, xk_t, kt_sb, 0, 1, "k"))
            put(0, 1, proj_job(wq_sb, xq_t, qt_sb, 0, 1, "q"))
            put(0, 4, proj_job(wq_sb, xq_t, qt_sb, 1, 1, "q"))
            put(0, 5, proj_job(wk_sb, xk_t, kt_sb, 1, 1, "k"))
            put(0, 8, proj_job(wk_sb, xk_t, kt_sb, 2, 1, "k"))
            put(0, 12, proj_job(wk_sb, xk_t, kt_sb, 3, 1, "k"))
            for n in range(3):
                put(0, 13 + n, vjob(n))
            put(1, 0, vjob(3))
            for n in range(4):
                put(1, 1 + n, vjob(4 + n))
            for n in range(4):
                put(1, 5 + n, vjob(8 + n))
            for n in range(4):
                put(1, 9 + n, vjob(12 + n))
            put(1, 13, proj_job(wq_sb, xq_t, qt_sb, 2, 0, "q"))
            put(1, 14, proj_job(wq_sb, xq_t, qt_sb, 3, 0, "q"))
            put(1, 15, proj_job(wq_sb, xq_t, qt_sb, 2, 1, "q"))
            put(2, 0, proj_job(wq_sb, xq_t, qt_sb, 3, 1, "q"))
            opos = [(5, 2), (5, 7), (5, 12), (6, 2), (6, 7), (6, 12),
                    (7, 2), (7, 7)]
            for eb in range(EB):
                put(*opos[eb], oproj_eb(0, eb))

            def normalize(h, qh, oacc, recip, i):
                jb = h // 2
                off = DH * (h % 2)
                q0 = qh * 1024
                bc = psc.tile([DH, 1024], F32, tag="sc", name=f"bc{i % 2}")
                for n in range(2):
                    nc.tensor.matmul(
                        bc[:, n * 512:(n + 1) * 512],
                        ones64[:],
                        recip[:, n * 512:(n + 1) * 512],
                        start=True,
                        stop=True,
                    )
                bcast = npool.tile([DH, 1024], F16, tag="bcast",
                                   name=f"bst{i % 2}")
                nc.vector.tensor_copy(bcast[:], bc[:])
                nc.vector.tensor_mul(
                    ao_sb[off:off + DH, jb, q0:q0 + 1024],
                    oacc[0:DH, :],
                    bcast[:],
                )

            # pre-task projections: first scores need Q sch0-1 + K sch0 (jb0)
            proj_job(wk_sb, xk_t, kt_sb, 0, 0, "k")()
            proj_job(wq_sb, xq_t, qt_sb, 0, 0, "q")()
            proj_job(wq_sb, xq_t, qt_sb, 1, 0, "q")()

            tasks = [(qh, h) for qh in range(QH) for h in range(HL)]
            prev = None  # (qh, h, ex dict) of the task whose attn@V is due
            for i in range(len(tasks) + 1):
                cur = None
                if i < len(tasks):
                    qh, h = tasks[i]
                    q0 = qh * 1024
                    jb = h // 2
                    off = DH * (h % 2)
                    cur = (qh, h, {})
                oacc = None
                oacc7 = None
                if prev is not None:
                    oacc = poacc.tile([DH + 1, 1024], F32, tag="oacc",
                                      name=f"oacc{i % 2}")
                if i == len(tasks) - 1:
                    oacc7 = poacc.tile([DH + 1, 1024], F32, tag="oacc",
                                       name="oacc7")
                for kb in range(KB):
                    if cur is not None:
                        sc = psc.tile([P, 1024], F32, tag="sc",
                                      name=f"sc{kb % 2}")
                        for n in range(2):
                            nc.tensor.matmul(
                                sc[:, n * 512:(n + 1) * 512],
                                kt_sb[off:off + DH, jb, kb * P:(kb + 1) * P],
                                qt_sb[off:off + DH, jb,
                                      q0 + n * 512:q0 + (n + 1) * 512],
                                start=True,
                                stop=True,
                            )
                        cur[2][kb] = expp.tile([P, 1024], F16, tag="ex",
                                               name=f"ex{(i * KB + kb) % 17}")
                        nc.scalar.activation(cur[2][kb][:], sc[:], AF.Exp)
                    if prev is not None:
                        ph = prev[1]
                        pex = prev[2][kb]
                        for n in range(2):
                            nc.tensor.matmul(
                                oacc[:, n * 512:(n + 1) * 512],
                                v_sb[:, kb, ph, :],
                                pex[:, n * 512:(n + 1) * 512],
                                start=(kb == 0),
                                stop=(kb == KB - 1),
                            )
                    if oacc7 is not None and kb >= 1:
                        for n in range(2):
                            nc.tensor.matmul(
                                oacc7[:, n * 512:(n + 1) * 512],
                                v_sb[:, kb - 1, h, :],
                                cur[2][kb - 1][:, n * 512:(n + 1) * 512],
                                start=(kb == 1),
                                stop=False,
                            )
                    for fn in fillers.pop((i, kb), ()):
                        fn()
                if prev is not None:
                    recip = npool.tile([1, 1024], F32R, tag="recip",
                                       name=f"recip{i % 2}")
                    with nc.allow_low_precision(reason="fp32r softmax denom"):
                        nc.vector.reciprocal(recip[:], oacc[DH:DH + 1, :])
                    normalize(prev[1], prev[0], oacc, recip, i)
                if oacc7 is not None:
                    # last task consumed its own exps lag-1; finish kb15 and
                    # normalize so only the qh1 oproj remains in the tail
                    for n in range(2):
                        nc.tensor.matmul(
                            oacc7[:, n * 512:(n + 1) * 512],
                            v_sb[:, KB - 1, h, :],
                            cur[2][KB - 1][:, n * 512:(n + 1) * 512],
                            start=False,
                            stop=True,
                        )
                    recip7 = npool.tile([1, 1024], F32R, tag="recip",
                                        name="recip7")
                    with nc.allow_low_precision(reason="fp32r softmax denom"):
                        nc.vector.reciprocal(recip7[:], oacc7[DH:DH + 1, :])
                    normalize(h, qh, oacc7, recip7, 99)
                    prev = None
                    break
                prev = cur

            for eb in range(EB):
                oproj_eb(1, eb, evac="mix", tailpool=(eb % 2 == 1))()

    nc.finalize()
    return nc


_NC_CACHE = None


def _get_nc():
    global _NC_CACHE
    if _NC_CACHE is None:
        _NC_CACHE = build_nc()
    return _NC_CACHE


def make_in_maps(query, key, value, Wq, Wk, Wv, Wo):
    """Build the 8 per-core input dicts from the full tensors."""
    query = np.asarray(query, np.float32)
    key = np.asarray(key, np.float32)
    value = np.asarray(value, np.float32)
    Wq = np.asarray(Wq, np.float32)
    Wk = np.asarray(Wk, np.float32)
    Wv = np.asarray(Wv, np.float32)
    Wo = np.asarray(Wo, np.float32)

    def pmajor(a2d, inner):  # [Drows, inner] -> [P, Drows//P, inner]
        return np.ascontiguousarray(
            a2d.reshape(KD, P, inner).transpose(1, 0, 2)
        )

    def schunk(a2d):  # X^T [D, S] -> [P, NS, KD, 512]
        return np.ascontiguousarray(
            a2d.reshape(KD, P, 4, 512).transpose(1, 2, 0, 3)
        )

    scale = np.float32(1.0 / np.sqrt(DH))
    xs = {}
    for b in range(B):
        xs[b] = {
            "xq": schunk(np.ascontiguousarray(query[b].T)).astype(np.float16),
            "xk": schunk(np.ascontiguousarray(key[b].T)).astype(np.float16),
            "xv": schunk(np.ascontiguousarray(value[b].T)).astype(np.float16),
        }
    ws = {}
    for hg in range(4):
        sl = slice(hg * J, (hg + 1) * J)
        wo_t = np.ascontiguousarray(Wo[:, sl].T)  # [256, 1024]
        ws[hg] = {
            "wq": pmajor(np.ascontiguousarray(Wq[sl].T * scale), J).astype(
                np.float16),
            "wk": pmajor(np.ascontiguousarray(Wk[sl].T), J).astype(np.float16),
            "wv": pmajor(np.ascontiguousarray(Wv[sl].T), J).astype(np.float16),
            "wo": np.ascontiguousarray(
                wo_t.reshape(2, P, D).transpose(1, 0, 2)
            ).astype(np.float16),
        }
    in_maps = []
    for c in range(NCORES):
        b, hg = c // 4, c % 4
        m = {}
        m.update(xs[b])
        m.update(ws[hg])
        in_maps.append(m)
    return in_maps


def assemble(results, bo):
    """Sum the 4 per-core partials per batch, add bo."""
    bo = np.asarray(bo, np.float32)
    out = np.zeros((B, S, D), np.float32)
    for c in range(NCORES):
        b = c // 4
        part = results[c]["out_t"].astype(np.float32).reshape(D, S).T
        out[b] += part
    out += bo[None, None, :]
    return out


def kernel(query, key, value, Wq, Wk, Wv, Wo, bo):
    import os
    import time

    os.environ.setdefault("NEURON_RT_RESET_CORES", "1")
    from concourse.bass_utils import run_bass_kernel_spmd

    nc = _get_nc()
    in_maps = make_in_maps(query, key, value, Wq, Wk, Wv, Wo)
    last_exc = None
    for attempt in range(3):
        try:
            res = run_bass_kernel_spmd(nc, in_maps, list(range(NCORES)))
            return assemble(res.results, bo)
        except Exception as e:  # transient NRT_EXEC_UNIT_UNRECOVERABLE etc.
            last_exc = e
            time.sleep(2.0)
    raise last_exc


# revision 17
# speedup vs baseline: 1.0914x; 1.0029x over previous
"""Multi-head attention (B=2, S=2048, D=1024, H=16, Dh=64) on 8 TRN2 cores.

Sharding: data-parallel over batch (2) x tensor-parallel over heads (16 -> 4
groups of 4). Core c handles batch c//4, heads [4*(c%4), 4*(c%4)+4).
Each core computes its partial output projection (Wo column slice); the host
sums the 4 per-core partials per batch (the "all-reduce") and adds bo.

All-fp16 data path (fp8/DoubleRow was tried and is numerically dead here:
quantization noise on scores/weights/values does NOT average down — the
attention output is a weighted mean whose magnitude shrinks as fast as the
noise, so fp8 anywhere in the value path lands at ~3-7% output error vs the
2e-2 gate; fp16 gives ~9e-4).

The kernel is PE-bound (fp16 matmul floor ~401k PE cycles = 167us at the
2.4GHz max p-state vs ACT's 133us exp stream). The TRN2 p-state model
punishes recurring PE dependency stalls (they reset the clock ramp and lock
the PE at 1.2GHz), and the cost model serializes all DMA transfers on one
track (~2.9us per 0.5MB s-chunk), which shapes the whole schedule:

  - x streams in s-chunks [P, KD, 512] in exact consumption order on one
    queue; every projection job is runnable right after one chunk lands.
  - V is projected directly in natural [s, j] layout (x chunk stationary,
    Wv moving): no PE transposes of V^T.
  - attn@V(i) runs one full task LATER (task-lag): it reads only buffered
    exp tiles + V, so the PE never stalls on the live exp stream. Only the
    last task consumes its own exps lag-1, and it compresses attn@V(6)
    into its first half so only norm(7) + the qh1 output projection remain
    after the final exp.
  - Filler jobs (jb1/late projections, V s-blocks, qh0 output projection)
    are placed at hand-tuned (task, kb) slots to keep the PE dense.
"""

import numpy as np
from contextlib import ExitStack

import concourse.bass as bass
from concourse import bacc
import concourse.mybir as mybir
import concourse.tile as tile

F32 = mybir.dt.float32
F32R = mybir.dt.float32r
F16 = mybir.dt.float16
AF = mybir.ActivationFunctionType

B = 2
S = 2048
D = 1024
H = 16
DH = 64
NCORES = 8
HL = 4          # heads per core
J = HL * DH     # 256 local projection width
P = 128
KD = D // P     # 8 d-chunks
KB = S // P     # 16 k-blocks
QH = S // 1024  # 2 q-halves of 1024
EB = D // P     # 8 e-blocks
NS = 4          # s-chunks of 512


def build_nc():
    nc = bacc.Bacc()

    xq = nc.dram_tensor("xq", [P, NS, KD, 512], F16, kind="ExternalInput")
    xk = nc.dram_tensor("xk", [P, NS, KD, 512], F16, kind="ExternalInput")
    xv = nc.dram_tensor("xv", [P, NS, KD, 512], F16, kind="ExternalInput")
    wq = nc.dram_tensor("wq", [P, KD, J], F16, kind="ExternalInput")
    wk = nc.dram_tensor("wk", [P, KD, J], F16, kind="ExternalInput")
    wv = nc.dram_tensor("wv", [P, KD, J], F16, kind="ExternalInput")
    wo = nc.dram_tensor("wo", [P, 2, D], F16, kind="ExternalInput")
    out_t = nc.dram_tensor("out_t", [EB, P, S], F16, kind="ExternalOutput")

    with tile.TileContext(nc) as tc, ExitStack() as st:
        const = st.enter_context(tc.tile_pool(name="const", bufs=1))
        persist = st.enter_context(tc.tile_pool(name="persist", bufs=1))
        xpool = st.enter_context(tc.tile_pool(name="xstream", bufs=12))

        wq_sb = const.tile([P, KD, J], F16, tag="wq")
        wk_sb = const.tile([P, KD, J], F16, tag="wk")
        wv_sb = const.tile([P, KD, J], F16, tag="wv")
        wo_sb = const.tile([P, 2, D], F16, tag="wo")

        qt_sb = persist.tile([P, 2, S], F16, tag="qt")   # Q_T [256, 2048]
        kt_sb = persist.tile([P, 2, S], F16, tag="kt")   # K_T
        # V natural layout + per-head ones col: [s_part, kb, h, 65]
        v_sb = persist.tile([P, KB, HL, DH + 1], F16, tag="v")
        ao_sb = persist.tile([P, 2, S], F16, tag="ao")   # normalized attn out ^T

        ones64 = const.tile([1, DH], F32R, tag="ones64")

        # --- DMA: one serial stream; order = consumption order -----------
        xq_t = [xpool.tile([P, KD, 512], F16, tag="xc", name=f"xq{c}")
                for c in range(NS)]
        xk_t = [xpool.tile([P, KD, 512], F16, tag="xc", name=f"xk{c}")
                for c in range(NS)]
        xv_t = [xpool.tile([P, KD, 512], F16, tag="xc", name=f"xv{c}")
                for c in range(NS)]
        ones16 = nc.inline_tensor(np.ones((P, KB * HL), np.float16),
                                  name="ones16")
        ones_f32 = nc.inline_tensor(np.ones((1, DH), np.float32), name="ones_f")
        dma_order = [
            (wk_sb, wk, None),
            (xk_t[0], xk, 0), (wq_sb, wq, None),
            (xq_t[0], xq, 0), (xq_t[1], xq, 1),
            (xk_t[1], xk, 1), (xk_t[2], xk, 2), (xk_t[3], xk, 3),
            (wv_sb, wv, None),
            (xv_t[0], xv, 0), (xq_t[2], xq, 2), (xv_t[1], xv, 1),
            (xq_t[3], xq, 3), (xv_t[2], xv, 2), (xv_t[3], xv, 3),
        ]
        for dst, srcd, sch in dma_order:
            if sch is None:
                nc.sync.dma_start(out=dst[:], in_=srcd[:])
            else:
                nc.sync.dma_start(out=dst[:], in_=srcd[:, sch])
        nc.sync.dma_start(out=v_sb[:, :, :, DH], in_=ones16.ap())
        nc.sync.dma_start(out=ones64[:], in_=ones_f32.ap().bitcast(F32R))
        nc.sync.dma_start(out=wo_sb[:], in_=wo[:])

        # --- attention pipeline with interleaved filler work --------------
        with tc.tile_pool(name="psc", bufs=2, space="PSUM") as psc, tc.tile_pool(
            name="poacc", bufs=2, space="PSUM"
        ) as poacc, tc.tile_pool(name="expp", bufs=17) as expp, tc.tile_pool(
            name="npool", bufs=2
        ) as npool, tc.tile_pool(name="ostage", bufs=4) as opool:

            def proj_job(wsb, xts, dst, sch, jb, nm):
                def f():
                    pp = psc.tile([P, 512], F32, tag="sc",
                                  name=f"pj{nm}{sch}{jb}")
                    for c in range(KD):
                        nc.tensor.matmul(
                            pp[:, :512],
                            wsb[:, c, jb * P:(jb + 1) * P],
                            xts[sch][:, c, :],
                            start=(c == 0),
                            stop=(c == KD - 1),
                        )
                    nc.vector.tensor_copy(
                        dst[:, jb, sch * 512:(sch + 1) * 512], pp[:])
                return f

            def vjob(sb):
                def f():
                    vp = psc.tile([P, J], F32, tag="sc", name=f"vp{sb}")
                    for c in range(KD):
                        nc.tensor.matmul(
                            vp[:, :J],
                            xv_t[sb // 4][:, c, (sb % 4) * P:(sb % 4 + 1) * P],
                            wv_sb[:, c, :],
                            start=(c == 0),
                            stop=(c == KD - 1),
                        )
                    nc.vector.tensor_copy(v_sb[:, sb, :, 0:DH], vp[:])
                return f

            def oproj_eb(qh, eb, evac="dve", tailpool=False):
                def f():
                    q0 = qh * 1024
                    ob = opool.tile([P, 1024], F16, tag="ob",
                                    name=f"ob{(qh * EB + eb) % 4}")
                    for stl in range(2):
                        s0 = q0 + stl * 512
                        pool = poacc if (tailpool and stl == 1) else psc
                        tg = "oacc" if (tailpool and stl == 1) else "sc"
                        po = pool.tile([P, 512], F32, tag=tg,
                                       name=f"po_{qh}_{eb}_{stl}")
                        for jbx in range(2):
                            nc.tensor.matmul(
                                po[:, :512],
                                wo_sb[:, jbx, eb * P:(eb + 1) * P],
                                ao_sb[:, jbx, s0:s0 + 512],
                                start=(jbx == 0),
                                stop=(jbx == 1),
                            )
                        d = ob[:, stl * 512:(stl + 1) * 512]
                        if evac == "act" or (evac == "mix" and stl == 0):
                            nc.scalar.copy(d, po[:, :512])
                        else:
                            nc.vector.tensor_copy(d, po[:, :512])
                    nc.sync.dma_start(out=out_t[eb][:, q0:q0 + 1024],
                                      in_=ob[:])
                return f

            # filler schedule: fillers[(task, kb)] run inside the kb loop
            # after scores/exp/attn@V; positions tuned against the model
            fillers = {}

            def put(i, kb, fn):
                fillers.setdefault((i, kb), []).append(fn)

            put(0, 3, proj_job(wk_sb, xk_t, kt_sb, 1, 0, "k"))
            put(0, 7, proj_job(wk_sb, xk_t, kt_sb, 2, 0, "k"))
            put(0, 10, proj_job(wk_sb, xk_t, kt_sb, 3, 0, "k"))
            put(0, 0, proj_job(wk_sb, xk_t, kt_sb, 0, 1, "k"))
            put(0, 1, proj_job(wq_sb, xq_t, qt_sb, 0, 1, "q"))
            put(0, 4, proj_job(wq_sb, xq_t, qt_sb, 1, 1, "q"))
            put(0, 5, proj_job(wk_sb, xk_t, kt_sb, 1, 1, "k"))
            put(0, 8, proj_job(wk_sb, xk_t, kt_sb, 2, 1, "k"))
            put(0, 12, proj_job(wk_sb, xk_t, kt_sb, 3, 1, "k"))
            for n in range(3):
                put(0, 13 + n, vjob(n))
            put(1, 0, vjob(3))
            for n in range(4):
                put(1, 1 + n, vjob(4 + n))
            for n in range(4):
                put(1, 5 + n, vjob(8 + n))
            for n in range(4):
                put(1, 9 + n, vjob(12 + n))
            put(1, 13, proj_job(wq_sb, xq_t, qt_sb, 2, 0, "q"))
            put(1, 14, proj_job(wq_sb, xq_t, qt_sb, 3, 0, "q"))
            put(1, 15, proj_job(wq_sb, xq_t, qt_sb, 2, 1, "q"))
            put(2, 0, proj_job(wq_sb, xq_t, qt_sb, 3, 1, "q"))
            opos = [(5, 2), (5, 7), (5, 12), (6, 2), (6, 7), (6, 12),
                    (7, 2), (7, 7)]
            for eb in range(EB):
                put(*opos[eb], oproj_eb(0, eb))

            def normalize(h, q0, w, oacc, recip, nm):
                def f():
                    jbn = h // 2
                    off = DH * (h % 2)
                    bc = psc.tile([DH, w], F32, tag="sc", name=f"bc{nm}")
                    for n in range(w // 512):
                        nc.tensor.matmul(
                            bc[:, n * 512:(n + 1) * 512],
                            ones64[:],
                            recip[:, n * 512:(n + 1) * 512],
                            start=True,
                            stop=True,
                        )
                    bcast = npool.tile([DH, w], F16, tag="bcast",
                                       name=f"bst{nm}")
                    nc.vector.tensor_copy(bcast[:], bc[:])
                    nc.vector.tensor_mul(
                        ao_sb[off:off + DH, jbn, q0:q0 + w],
                        oacc[0:DH, :],
                        bcast[:],
                    )
                return f

            # pre-task projections: first scores need Q sch0-1 + K sch0 (jb0)
            proj_job(wk_sb, xk_t, kt_sb, 0, 0, "k")()
            proj_job(wq_sb, xq_t, qt_sb, 0, 0, "q")()
            proj_job(wq_sb, xq_t, qt_sb, 1, 0, "q")()

            # attn@V(i) runs one full task later (task-lag): it reads only
            # buffered exps + V, so the PE never stalls on the live exp
            # stream (a per-kb dependency stall would reset the clock ramp
            # and lock the PE at 1.2GHz). The last task additionally
            # consumes its own exps lag-1 and compresses attn@V(6) into
            # its first half so the tail is just norm(7) + the qh1 oproj.
            tasks = [(qh, hh) for qh in range(QH) for hh in range(HL)]
            NT = len(tasks)

            def attn_v(oacc, hh, ex, kb):
                for n in range(2):
                    nc.tensor.matmul(
                        oacc[:, n * 512:(n + 1) * 512],
                        v_sb[:, kb, hh, :],
                        ex[kb][:, n * 512:(n + 1) * 512],
                        start=(kb == 0),
                        stop=(kb == KB - 1),
                    )

            prev = None  # (qh, h, ex) whose attn@V runs this iter
            exn = 0
            for i in range(NT):
                qh, h = tasks[i]
                q0 = qh * 1024
                jb = h // 2
                off = DH * (h % 2)
                cur = (qh, h, {})
                oacc = poacc.tile([DH + 1, 1024], F32, tag="oacc",
                                  name=f"oacc{i % 2}") if prev else None
                oacc7 = poacc.tile([DH + 1, 1024], F32, tag="oacc",
                                   name="oacc7") if i == NT - 1 else None
                for kb in range(KB):
                    sc = psc.tile([P, 1024], F32, tag="sc", name=f"sc{kb % 2}")
                    for n in range(2):
                        nc.tensor.matmul(
                            sc[:, n * 512:(n + 1) * 512],
                            kt_sb[off:off + DH, jb, kb * P:(kb + 1) * P],
                            qt_sb[off:off + DH, jb,
                                  q0 + n * 512:q0 + (n + 1) * 512],
                            start=True,
                            stop=True,
                        )
                    cur[2][kb] = expp.tile([P, 1024], F16, tag="ex",
                                           name=f"ex{exn % 17}")
                    exn += 1
                    nc.scalar.activation(cur[2][kb][:], sc[:], AF.Exp)
                    if prev is not None:
                        if oacc7 is not None:
                            if kb < KB // 2:
                                attn_v(oacc, prev[1], prev[2], 2 * kb)
                                attn_v(oacc, prev[1], prev[2], 2 * kb + 1)
                        else:
                            attn_v(oacc, prev[1], prev[2], kb)
                    if oacc7 is not None and kb >= 1:
                        attn_v(oacc7, h, cur[2], kb - 1)
                    if oacc7 is not None and kb == KB // 2:
                        recip6 = npool.tile([1, 1024], F32R, tag="recip",
                                            name="recip6")
                        with nc.allow_low_precision(reason="softmax denom"):
                            nc.vector.reciprocal(recip6[:],
                                                 oacc[DH:DH + 1, :])
                    if oacc7 is not None and kb == KB // 2 + 2:
                        normalize(prev[1], prev[0] * 1024, 1024, oacc,
                                  recip6, "n6")()
                    for fn in fillers.pop((i, kb), ()):
                        fn()
                if prev is not None and oacc7 is None:
                    recip = npool.tile([1, 1024], F32R, tag="recip",
                                       name=f"recip{i % 2}")
                    with nc.allow_low_precision(reason="fp32r softmax denom"):
                        nc.vector.reciprocal(recip[:], oacc[DH:DH + 1, :])
                    normalize(prev[1], prev[0] * 1024, 1024, oacc, recip,
                              f"n{i % 2}")()
                if oacc7 is not None:
                    attn_v(oacc7, h, cur[2], KB - 1)
                    recip7 = npool.tile([1, 1024], F32R, tag="recip",
                                        name="recip7")
                    with nc.allow_low_precision(reason="fp32r softmax denom"):
                        nc.vector.reciprocal(recip7[:], oacc7[DH:DH + 1, :])
                    normalize(h, q0, 1024, oacc7, recip7, "nd")()
                    prev = None
                else:
                    prev = cur

            # tail: qh1 output projection, po tiles spread over both pools
            for eb in range(EB):
                oproj_eb(1, eb, evac="mix", tailpool=True)()

    nc.finalize()
    return nc


_NC_CACHE = None


def _get_nc():
    global _NC_CACHE
    if _NC_CACHE is None:
        _NC_CACHE = build_nc()
    return _NC_CACHE


def make_in_maps(query, key, value, Wq, Wk, Wv, Wo):
    """Build the 8 per-core input dicts from the full tensors."""
    query = np.asarray(query, np.float32)
    key = np.asarray(key, np.float32)
    value = np.asarray(value, np.float32)
    Wq = np.asarray(Wq, np.float32)
    Wk = np.asarray(Wk, np.float32)
    Wv = np.asarray(Wv, np.float32)
    Wo = np.asarray(Wo, np.float32)

    def pmajor(a2d, inner):  # [Drows, inner] -> [P, Drows//P, inner]
        return np.ascontiguousarray(
            a2d.reshape(KD, P, inner).transpose(1, 0, 2)
        )

    def schunk(a2d):  # X^T [D, S] -> [P, NS, KD, 512]
        return np.ascontiguousarray(
            a2d.reshape(KD, P, NS, 512).transpose(1, 2, 0, 3)
        )

    scale = np.float32(1.0 / np.sqrt(DH))
    xs = {}
    for b in range(B):
        xs[b] = {
            "xq": schunk(np.ascontiguousarray(query[b].T)).astype(np.float16),
            "xk": schunk(np.ascontiguousarray(key[b].T)).astype(np.float16),
            "xv": schunk(np.ascontiguousarray(value[b].T)).astype(np.float16),
        }
    ws = {}
    for hg in range(4):
        sl = slice(hg * J, (hg + 1) * J)
        wo_t = np.ascontiguousarray(Wo[:, sl].T)  # [256, 1024]
        ws[hg] = {
            "wq": pmajor(np.ascontiguousarray(Wq[sl].T * scale), J).astype(
                np.float16),
            "wk": pmajor(np.ascontiguousarray(Wk[sl].T), J).astype(np.float16),
            "wv": pmajor(np.ascontiguousarray(Wv[sl].T), J).astype(np.float16),
            "wo": np.ascontiguousarray(
                wo_t.reshape(2, P, D).transpose(1, 0, 2)
            ).astype(np.float16),
        }
    in_maps = []
    for c in range(NCORES):
        b, hg = c // 4, c % 4
        m = {}
        m.update(xs[b])
        m.update(ws[hg])
        in_maps.append(m)
    return in_maps


def assemble(results, bo):
    """Sum the 4 per-core partials per batch, add bo."""
    bo = np.asarray(bo, np.float32)
    out = np.zeros((B, S, D), np.float32)
    for c in range(NCORES):
        b = c // 4
        part = results[c]["out_t"].astype(np.float32).reshape(D, S).T
        out[b] += part
    out += bo[None, None, :]
    return out


def kernel(query, key, value, Wq, Wk, Wv, Wo, bo):
    import os
    import time

    os.environ.setdefault("NEURON_RT_RESET_CORES", "1")
    from concourse.bass_utils import run_bass_kernel_spmd

    nc = _get_nc()
    in_maps = make_in_maps(query, key, value, Wq, Wk, Wv, Wo)
    last_exc = None
    for attempt in range(3):
        try:
            res = run_bass_kernel_spmd(nc, in_maps, list(range(NCORES)))
            return assemble(res.results, bo)
        except Exception as e:  # transient NRT_EXEC_UNIT_UNRECOVERABLE etc.
            last_exc = e
            time.sleep(2.0)
    raise last_exc


# revision 22
# speedup vs baseline: 1.0939x; 1.0023x over previous
"""Multi-head attention (B=2, S=2048, D=1024, H=16, Dh=64) on 8 TRN2 cores.

Sharding: data-parallel over batch (2) x tensor-parallel over heads (16 -> 4
groups of 4). Core c handles batch c//4, heads [4*(c%4), 4*(c%4)+4).
Each core computes its partial output projection (Wo column slice); the host
sums the 4 per-core partials per batch (the "all-reduce") and adds bo.

All-fp16 data path (fp8/DoubleRow was tried and is numerically dead here:
quantization noise on scores/weights/values does NOT average down — the
attention output is a weighted mean whose magnitude shrinks as fast as the
noise, so fp8 anywhere in the value path lands at ~3-7% output error vs the
2e-2 gate; fp16 gives ~9e-4).

The kernel is PE-bound (fp16 matmul floor ~401k PE cycles = 167us at the
2.4GHz max p-state vs ACT's 133us exp stream). The TRN2 p-state model
punishes recurring PE dependency stalls (they reset the clock ramp and lock
the PE at 1.2GHz), and the cost model serializes all DMA transfers on one
track (~2.9us per 0.5MB s-chunk), which shapes the whole schedule:

  - x streams in s-chunks [P, KD, 512] in exact consumption order on one
    queue; every projection job is runnable right after one chunk lands.
  - V is projected directly in natural [s, j] layout (x chunk stationary,
    Wv moving): no PE transposes of V^T.
  - attn@V(i) runs one full task LATER (task-lag): it reads only buffered
    exp tiles + V, so the PE never stalls on the live exp stream. Only the
    last task consumes its own exps lag-1, and it compresses attn@V(6)
    into its first half so only norm(7) + the qh1 output projection remain
    after the final exp.
  - Filler jobs (jb1/late projections, V s-blocks, qh0 output projection)
    are placed at hand-tuned (task, kb) slots to keep the PE dense.
"""

import numpy as np
from contextlib import ExitStack

import concourse.bass as bass
from concourse import bacc
import concourse.mybir as mybir
import concourse.tile as tile

F32 = mybir.dt.float32
F32R = mybir.dt.float32r
F16 = mybir.dt.float16
AF = mybir.ActivationFunctionType

B = 2
S = 2048
D = 1024
H = 16
DH = 64
NCORES = 8
HL = 4          # heads per core
J = HL * DH     # 256 local projection width
P = 128
KD = D // P     # 8 d-chunks
KB = S // P     # 16 k-blocks
QH = S // 1024  # 2 q-halves of 1024
EB = D // P     # 8 e-blocks
NS = 4          # s-chunks of 512


def build_nc():
    nc = bacc.Bacc()

    xq = nc.dram_tensor("xq", [P, NS, KD, 512], F16, kind="ExternalInput")
    xk = nc.dram_tensor("xk", [P, NS, KD, 512], F16, kind="ExternalInput")
    xv = nc.dram_tensor("xv", [P, NS, KD, 512], F16, kind="ExternalInput")
    wq = nc.dram_tensor("wq", [P, KD, J], F16, kind="ExternalInput")
    wk = nc.dram_tensor("wk", [P, KD, J], F16, kind="ExternalInput")
    wv = nc.dram_tensor("wv", [P, KD, J], F16, kind="ExternalInput")
    wo = nc.dram_tensor("wo", [P, 2, D], F16, kind="ExternalInput")
    out_t = nc.dram_tensor("out_t", [EB, P, S], F16, kind="ExternalOutput")

    with tile.TileContext(nc) as tc, ExitStack() as st:
        const = st.enter_context(tc.tile_pool(name="const", bufs=1))
        persist = st.enter_context(tc.tile_pool(name="persist", bufs=1))
        xpool = st.enter_context(tc.tile_pool(name="xstream", bufs=12))

        wq_sb = const.tile([P, KD, J], F16, tag="wq")
        wk_sb = const.tile([P, KD, J], F16, tag="wk")
        wv_sb = const.tile([P, KD, J], F16, tag="wv")
        wo_sb = const.tile([P, 2, D], F16, tag="wo")

        qt_sb = persist.tile([P, 2, S], F16, tag="qt")   # Q_T [256, 2048]
        kt_sb = persist.tile([P, 2, S], F16, tag="kt")   # K_T
        # V natural layout + per-head ones col: [s_part, kb, h, 65]
        v_sb = persist.tile([P, KB, HL, DH + 1], F16, tag="v")
        ao_sb = persist.tile([P, 2, S], F16, tag="ao")   # normalized attn out ^T

        ones64 = const.tile([1, DH], F32R, tag="ones64")

        # --- DMA: one serial stream; order = consumption order -----------
        xq_t = [xpool.tile([P, KD, 512], F16, tag="xc", name=f"xq{c}")
                for c in range(NS)]
        xk_t = [xpool.tile([P, KD, 512], F16, tag="xc", name=f"xk{c}")
                for c in range(NS)]
        xv_t = [xpool.tile([P, KD, 512], F16, tag="xc", name=f"xv{c}")
                for c in range(NS)]
        ones16 = nc.inline_tensor(np.ones((P, KB * HL), np.float16),
                                  name="ones16")
        ones_f32 = nc.inline_tensor(np.ones((1, DH), np.float32), name="ones_f")
        dma_order = [
            (wk_sb, wk, None),
            (xk_t[0], xk, 0), (wq_sb, wq, None),
            (xq_t[0], xq, 0), (xq_t[1], xq, 1),
            (xk_t[1], xk, 1), (xk_t[2], xk, 2), (xk_t[3], xk, 3),
            (wv_sb, wv, None),
            (xv_t[0], xv, 0), (xv_t[1], xv, 1), (xv_t[2], xv, 2),
            (xv_t[3], xv, 3), (xq_t[2], xq, 2), (xq_t[3], xq, 3),
        ]
        for dst, srcd, sch in dma_order:
            if sch is None:
                nc.sync.dma_start(out=dst[:], in_=srcd[:])
            else:
                nc.sync.dma_start(out=dst[:], in_=srcd[:, sch])
        nc.sync.dma_start(out=v_sb[:, :, :, DH], in_=ones16.ap())
        nc.sync.dma_start(out=ones64[:], in_=ones_f32.ap().bitcast(F32R))
        nc.sync.dma_start(out=wo_sb[:], in_=wo[:])

        # --- attention pipeline with interleaved filler work --------------
        with tc.tile_pool(name="psc", bufs=2, space="PSUM") as psc, tc.tile_pool(
            name="poacc", bufs=2, space="PSUM"
        ) as poacc, tc.tile_pool(name="expp", bufs=17) as expp, tc.tile_pool(
            name="npool", bufs=2
        ) as npool, tc.tile_pool(name="ostage", bufs=4) as opool:

            def proj_job(wsb, xts, dst, sch, jb, nm):
                def f():
                    pp = psc.tile([P, 512], F32, tag="sc",
                                  name=f"pj{nm}{sch}{jb}")
                    for c in range(KD):
                        nc.tensor.matmul(
                            pp[:, :512],
                            wsb[:, c, jb * P:(jb + 1) * P],
                            xts[sch][:, c, :],
                            start=(c == 0),
                            stop=(c == KD - 1),
                        )
                    nc.vector.tensor_copy(
                        dst[:, jb, sch * 512:(sch + 1) * 512], pp[:])
                return f

            def vjob(sb):
                def f():
                    vp = psc.tile([P, J], F32, tag="sc", name=f"vp{sb}")
                    for c in range(KD):
                        nc.tensor.matmul(
                            vp[:, :J],
                            xv_t[sb // 4][:, c, (sb % 4) * P:(sb % 4 + 1) * P],
                            wv_sb[:, c, :],
                            start=(c == 0),
                            stop=(c == KD - 1),
                        )
                    nc.vector.tensor_copy(v_sb[:, sb, :, 0:DH], vp[:])
                return f

            def oproj_eb(qh, eb, evac="dve", tailpool=False):
                def f():
                    q0 = qh * 1024
                    ob = opool.tile([P, 1024], F16, tag="ob",
                                    name=f"ob{(qh * EB + eb) % 4}")
                    for stl in range(2):
                        s0 = q0 + stl * 512
                        pool = poacc if (tailpool and stl == 1) else psc
                        tg = "oacc" if (tailpool and stl == 1) else "sc"
                        po = pool.tile([P, 512], F32, tag=tg,
                                       name=f"po_{qh}_{eb}_{stl}")
                        for jbx in range(2):
                            nc.tensor.matmul(
                                po[:, :512],
                                wo_sb[:, jbx, eb * P:(eb + 1) * P],
                                ao_sb[:, jbx, s0:s0 + 512],
                                start=(jbx == 0),
                                stop=(jbx == 1),
                            )
                        d = ob[:, stl * 512:(stl + 1) * 512]
                        if evac == "act" or (evac == "mix" and stl == 0):
                            nc.scalar.copy(d, po[:, :512])
                        else:
                            nc.vector.tensor_copy(d, po[:, :512])
                    nc.sync.dma_start(out=out_t[eb][:, q0:q0 + 1024],
                                      in_=ob[:])
                return f

            # filler schedule: fillers[(task, kb)] run inside the kb loop
            # after scores/exp/attn@V; positions tuned against the model
            fillers = {}

            def put(i, kb, fn):
                fillers.setdefault((i, kb), []).append(fn)

            put(0, 3, proj_job(wk_sb, xk_t, kt_sb, 1, 0, "k"))
            put(0, 7, proj_job(wk_sb, xk_t, kt_sb, 2, 0, "k"))
            put(0, 10, proj_job(wk_sb, xk_t, kt_sb, 3, 0, "k"))
            put(0, 0, proj_job(wk_sb, xk_t, kt_sb, 0, 1, "k"))
            put(0, 1, proj_job(wq_sb, xq_t, qt_sb, 0, 1, "q"))
            put(0, 4, proj_job(wq_sb, xq_t, qt_sb, 1, 1, "q"))
            put(0, 5, proj_job(wk_sb, xk_t, kt_sb, 1, 1, "k"))
            put(0, 8, proj_job(wk_sb, xk_t, kt_sb, 2, 1, "k"))
            put(0, 12, proj_job(wk_sb, xk_t, kt_sb, 3, 1, "k"))
            for n in range(3):
                put(0, 13 + n, vjob(n))
            put(1, 0, vjob(3))
            for n in range(4):
                put(1, 1 + n, vjob(4 + n))
            for n in range(4):
                put(1, 5 + n, vjob(8 + n))
            for n in range(4):
                put(1, 9 + n, vjob(12 + n))
            put(2, 3, proj_job(wq_sb, xq_t, qt_sb, 2, 0, "q"))
            put(2, 11, proj_job(wq_sb, xq_t, qt_sb, 3, 0, "q"))
            put(3, 3, proj_job(wq_sb, xq_t, qt_sb, 2, 1, "q"))
            put(3, 11, proj_job(wq_sb, xq_t, qt_sb, 3, 1, "q"))
            opos = [(5, 2), (5, 7), (5, 12), (6, 2), (6, 7), (6, 12),
                    (7, 2), (7, 7)]
            for eb in range(EB):
                put(*opos[eb], oproj_eb(0, eb))

            def normalize(h, q0, w, oacc, recip, nm):
                def f():
                    jbn = h // 2
                    off = DH * (h % 2)
                    bc = psc.tile([DH, w], F32, tag="sc", name=f"bc{nm}")
                    for n in range(w // 512):
                        nc.tensor.matmul(
                            bc[:, n * 512:(n + 1) * 512],
                            ones64[:],
                            recip[:, n * 512:(n + 1) * 512],
                            start=True,
                            stop=True,
                        )
                    bcast = npool.tile([DH, w], F16, tag="bcast",
                                       name=f"bst{nm}")
                    nc.vector.tensor_copy(bcast[:], bc[:])
                    nc.vector.tensor_mul(
                        ao_sb[off:off + DH, jbn, q0:q0 + w],
                        oacc[0:DH, :],
                        bcast[:],
                    )
                return f

            # pre-task projections: first scores need Q sch0-1 + K sch0 (jb0)
            proj_job(wk_sb, xk_t, kt_sb, 0, 0, "k")()
            proj_job(wq_sb, xq_t, qt_sb, 0, 0, "q")()
            proj_job(wq_sb, xq_t, qt_sb, 1, 0, "q")()

            # attn@V(i) runs one full task later (task-lag): it reads only
            # buffered exps + V, so the PE never stalls on the live exp
            # stream (a per-kb dependency stall would reset the clock ramp
            # and lock the PE at 1.2GHz). The last task additionally
            # consumes its own exps lag-1 and compresses attn@V(6) into
            # its first half so the tail is just norm(7) + the qh1 oproj.
            tasks = [(qh, hh) for qh in range(QH) for hh in range(HL)]
            NT = len(tasks)

            def attn_v(oacc, hh, ex, kb):
                for n in range(2):
                    nc.tensor.matmul(
                        oacc[:, n * 512:(n + 1) * 512],
                        v_sb[:, kb, hh, :],
                        ex[kb][:, n * 512:(n + 1) * 512],
                        start=(kb == 0),
                        stop=(kb == KB - 1),
                    )

            prev = None  # (qh, h, ex) whose attn@V runs this iter
            exn = 0
            for i in range(NT):
                qh, h = tasks[i]
                q0 = qh * 1024
                jb = h // 2
                off = DH * (h % 2)
                cur = (qh, h, {})
                oacc = poacc.tile([DH + 1, 1024], F32, tag="oacc",
                                  name=f"oacc{i % 2}") if prev else None
                oacc7 = poacc.tile([DH + 1, 1024], F32, tag="oacc",
                                   name="oacc7") if i == NT - 1 else None
                for kb in range(KB):
                    sc = psc.tile([P, 1024], F32, tag="sc", name=f"sc{kb % 2}")
                    for n in range(2):
                        nc.tensor.matmul(
                            sc[:, n * 512:(n + 1) * 512],
                            kt_sb[off:off + DH, jb, kb * P:(kb + 1) * P],
                            qt_sb[off:off + DH, jb,
                                  q0 + n * 512:q0 + (n + 1) * 512],
                            start=True,
                            stop=True,
                        )
                    cur[2][kb] = expp.tile([P, 1024], F16, tag="ex",
                                           name=f"ex{exn % 17}")
                    exn += 1
                    nc.scalar.activation(cur[2][kb][:], sc[:], AF.Exp)
                    if prev is not None:
                        if oacc7 is not None:
                            if kb < KB // 2:
                                attn_v(oacc, prev[1], prev[2], 2 * kb)
                                attn_v(oacc, prev[1], prev[2], 2 * kb + 1)
                        else:
                            attn_v(oacc, prev[1], prev[2], kb)
                    if oacc7 is not None and kb >= 1:
                        attn_v(oacc7, h, cur[2], kb - 1)
                    if oacc7 is not None and kb == KB // 2:
                        recip6 = npool.tile([1, 1024], F32R, tag="recip",
                                            name="recip6")
                        with nc.allow_low_precision(reason="softmax denom"):
                            nc.vector.reciprocal(recip6[:],
                                                 oacc[DH:DH + 1, :])
                    if oacc7 is not None and kb == KB // 2 + 2:
                        normalize(prev[1], prev[0] * 1024, 1024, oacc,
                                  recip6, "n6")()
                    for fn in fillers.pop((i, kb), ()):
                        fn()
                if prev is not None and oacc7 is None:
                    recip = npool.tile([1, 1024], F32R, tag="recip",
                                       name=f"recip{i % 2}")
                    with nc.allow_low_precision(reason="fp32r softmax denom"):
                        nc.vector.reciprocal(recip[:], oacc[DH:DH + 1, :])
                    normalize(prev[1], prev[0] * 1024, 1024, oacc, recip,
                              f"n{i % 2}")()
                if oacc7 is not None:
                    attn_v(oacc7, h, cur[2], KB - 1)
                    recip7 = npool.tile([1, 1024], F32R, tag="recip",
                                        name="recip7")
                    with nc.allow_low_precision(reason="fp32r softmax denom"):
                        nc.vector.reciprocal(recip7[:], oacc7[DH:DH + 1, :])
                    normalize(h, q0, 1024, oacc7, recip7, "nd")()
                    prev = None
                else:
                    prev = cur

            # tail: qh1 output projection, po tiles spread over both pools
            for eb in range(EB):
                oproj_eb(1, eb, evac="mix", tailpool=True)()

    nc.finalize()
    return nc


_NC_CACHE = None


def _get_nc():
    global _NC_CACHE
    if _NC_CACHE is None:
        _NC_CACHE = build_nc()
    return _NC_CACHE


def make_in_maps(query, key, value, Wq, Wk, Wv, Wo):
    """Build the 8 per-core input dicts from the full tensors."""
    query = np.asarray(query, np.float32)
    key = np.asarray(key, np.float32)
    value = np.asarray(value, np.float32)
    Wq = np.asarray(Wq, np.float32)
    Wk = np.asarray(Wk, np.float32)
    Wv = np.asarray(Wv, np.float32)
    Wo = np.asarray(Wo, np.float32)

    def pmajor(a2d, inner):  # [Drows, inner] -> [P, Drows//P, inner]
        return np.ascontiguousarray(
            a2d.reshape(KD, P, inner).transpose(1, 0, 2)
        )

    def schunk(a2d):  # X^T [D, S] -> [P, NS, KD, 512]
        return np.ascontiguousarray(
            a2d.reshape(KD, P, NS, 512).transpose(1, 2, 0, 3)
        )

    scale = np.float32(1.0 / np.sqrt(DH))
    xs = {}
    for b in range(B):
        xs[b] = {
            "xq": schunk(np.ascontiguousarray(query[b].T)).astype(np.float16),
            "xk": schunk(np.ascontiguousarray(key[b].T)).astype(np.float16),
            "xv": schunk(np.ascontiguousarray(value[b].T)).astype(np.float16),
        }
    ws = {}
    for hg in range(4):
        sl = slice(hg * J, (hg + 1) * J)
        wo_t = np.ascontiguousarray(Wo[:, sl].T)  # [256, 1024]
        ws[hg] = {
            "wq": pmajor(np.ascontiguousarray(Wq[sl].T * scale), J).astype(
                np.float16),
            "wk": pmajor(np.ascontiguousarray(Wk[sl].T), J).astype(np.float16),
            "wv": pmajor(np.ascontiguousarray(Wv[sl].T), J).astype(np.float16),
            "wo": np.ascontiguousarray(
                wo_t.reshape(2, P, D).transpose(1, 0, 2)
            ).astype(np.float16),
        }
    in_maps = []
    for c in range(NCORES):
        b, hg = c // 4, c % 4
        m = {}
        m.update(xs[b])
        m.update(ws[hg])
        in_maps.append(m)
    return in_maps


def assemble(results, bo):
    """Sum the 4 per-core partials per batch, add bo."""
    bo = np.asarray(bo, np.float32)
    out = np.zeros((B, S, D), np.float32)
    for c in range(NCORES):
        b = c // 4
        part = results[c]["out_t"].astype(np.float32).reshape(D, S).T
        out[b] += part
    out += bo[None, None, :]
    return out


def kernel(query, key, value, Wq, Wk, Wv, Wo, bo):
    import os
    import time

    os.environ.setdefault("NEURON_RT_RESET_CORES", "1")
    from concourse.bass_utils import run_bass_kernel_spmd

    nc = _get_nc()
    in_maps = make_in_maps(query, key, value, Wq, Wk, Wv, Wo)
    last_exc = None
    for attempt in range(3):
        try:
            res = run_bass_kernel_spmd(nc, in_maps, list(range(NCORES)))
            return assemble(res.results, bo)
        except Exception as e:  # transient NRT_EXEC_UNIT_UNRECOVERABLE etc.
            last_exc = e
            time.sleep(2.0)
    raise last_exc


# revision 28
# speedup vs baseline: 1.0955x; 1.0014x over previous
"""Multi-head attention (B=2, S=2048, D=1024, H=16, Dh=64) on 8 TRN2 cores.

Sharding: data-parallel over batch (2) x tensor-parallel over heads (16 -> 4
groups of 4). Core c handles batch c//4, heads [4*(c%4), 4*(c%4)+4).
Each core computes its partial output projection (Wo column slice); the host
sums the 4 per-core partials per batch (the "all-reduce") and adds bo.

All-fp16 data path (fp8/DoubleRow was tried and is numerically dead here:
quantization noise on scores/weights/values does NOT average down — the
attention output is a weighted mean whose magnitude shrinks as fast as the
noise, so fp8 anywhere in the value path lands at ~3-7% output error vs the
2e-2 gate; fp16 gives ~9e-4).

The kernel is PE-bound (fp16 matmul floor ~401k PE cycles = 167us at the
2.4GHz max p-state vs ACT's 133us exp stream). The TRN2 p-state model
punishes recurring PE dependency stalls (they reset the clock ramp and lock
the PE at 1.2GHz), and the cost model serializes all DMA transfers on one
track (~2.9us per 0.5MB s-chunk), which shapes the whole schedule:

  - x streams in s-chunks [P, KD, 512] in exact consumption order on one
    queue; every projection job is runnable right after one chunk lands.
  - V is projected directly in natural [s, j] layout (x chunk stationary,
    Wv moving): no PE transposes of V^T.
  - attn@V(i) runs one full task LATER (task-lag): it reads only buffered
    exp tiles + V, so the PE never stalls on the live exp stream. Only the
    last task consumes its own exps lag-1, and it compresses attn@V(6)
    into its first half so only norm(7) + the qh1 output projection remain
    after the final exp.
  - Filler jobs (jb1/late projections, V s-blocks, qh0 output projection)
    are placed at hand-tuned (task, kb) slots to keep the PE dense.
"""

import numpy as np
from contextlib import ExitStack

import concourse.bass as bass
from concourse import bacc
import concourse.mybir as mybir
import concourse.tile as tile

F32 = mybir.dt.float32
F32R = mybir.dt.float32r
F16 = mybir.dt.float16
AF = mybir.ActivationFunctionType

B = 2
S = 2048
D = 1024
H = 16
DH = 64
NCORES = 8
HL = 4          # heads per core
J = HL * DH     # 256 local projection width
P = 128
KD = D // P     # 8 d-chunks
KB = S // P     # 16 k-blocks
QH = S // 1024  # 2 q-halves of 1024
EB = D // P     # 8 e-blocks
NS = 4          # s-chunks of 512


def build_nc():
    nc = bacc.Bacc()

    xq = nc.dram_tensor("xq", [P, NS, KD, 512], F16, kind="ExternalInput")
    xk = nc.dram_tensor("xk", [P, NS, KD, 512], F16, kind="ExternalInput")
    xv = nc.dram_tensor("xv", [P, NS, KD, 512], F16, kind="ExternalInput")
    wq = nc.dram_tensor("wq", [P, KD, J], F16, kind="ExternalInput")
    wk = nc.dram_tensor("wk", [P, KD, J], F16, kind="ExternalInput")
    wv = nc.dram_tensor("wv", [P, KD, J], F16, kind="ExternalInput")
    wo = nc.dram_tensor("wo", [P, 2, D], F16, kind="ExternalInput")
    out_t = nc.dram_tensor("out_t", [EB, P, S], F16, kind="ExternalOutput")

    with tile.TileContext(nc) as tc, ExitStack() as st:
        const = st.enter_context(tc.tile_pool(name="const", bufs=1))
        persist = st.enter_context(tc.tile_pool(name="persist", bufs=1))
        xpool = st.enter_context(tc.tile_pool(name="xstream", bufs=12))

        wq_sb = const.tile([P, KD, J], F16, tag="wq")
        wk_sb = const.tile([P, KD, J], F16, tag="wk")
        wv_sb = const.tile([P, KD, J], F16, tag="wv")
        wo_sb = const.tile([P, 2, D], F16, tag="wo")

        qt_sb = persist.tile([P, 2, S], F16, tag="qt")   # Q_T [256, 2048]
        kt_sb = persist.tile([P, 2, S], F16, tag="kt")   # K_T
        # V natural layout + per-head ones col: [s_part, kb, h, 65]
        v_sb = persist.tile([P, KB, HL, DH + 1], F16, tag="v")
        ao_sb = persist.tile([P, 2, S], F16, tag="ao")   # normalized attn out ^T

        ones64 = const.tile([1, DH], F32R, tag="ones64")

        # --- DMA: one serial stream; order = consumption order -----------
        xq_t = [xpool.tile([P, KD, 512], F16, tag="xc", name=f"xq{c}")
                for c in range(NS)]
        xk_t = [xpool.tile([P, KD, 512], F16, tag="xc", name=f"xk{c}")
                for c in range(NS)]
        xv_t = [xpool.tile([P, KD, 512], F16, tag="xc", name=f"xv{c}")
                for c in range(NS)]
        ones16 = nc.inline_tensor(np.ones((P, KB * HL), np.float16),
                                  name="ones16")
        ones_f32 = nc.inline_tensor(np.ones((1, DH), np.float32), name="ones_f")
        dma_order = [
            (wk_sb, wk, None),
            (xk_t[0], xk, 0), (wq_sb, wq, None),
            (xq_t[0], xq, 0), (xq_t[1], xq, 1),
            (xk_t[1], xk, 1), (xk_t[2], xk, 2), (xk_t[3], xk, 3),
            (wv_sb, wv, None),
            (xv_t[0], xv, 0), (xv_t[1], xv, 1), (xv_t[2], xv, 2),
            (xv_t[3], xv, 3), (xq_t[2], xq, 2), (xq_t[3], xq, 3),
        ]
        for dst, srcd, sch in dma_order:
            if sch is None:
                nc.sync.dma_start(out=dst[:], in_=srcd[:])
            else:
                nc.sync.dma_start(out=dst[:], in_=srcd[:, sch])
        nc.sync.dma_start(out=v_sb[:, :, :, DH], in_=ones16.ap())
        nc.sync.dma_start(out=ones64[:], in_=ones_f32.ap().bitcast(F32R))
        nc.sync.dma_start(out=wo_sb[:], in_=wo[:])

        # --- attention pipeline with interleaved filler work --------------
        with tc.tile_pool(name="psc", bufs=2, space="PSUM") as psc, tc.tile_pool(
            name="poacc", bufs=2, space="PSUM"
        ) as poacc, tc.tile_pool(name="expp", bufs=21) as expp, tc.tile_pool(
            name="npool", bufs=2
        ) as npool, tc.tile_pool(name="ostage", bufs=4) as opool:

            def proj_job(wsb, xts, dst, sch, jb, nm):
                def f():
                    pp = psc.tile([P, 512], F32, tag="sc",
                                  name=f"pj{nm}{sch}{jb}")
                    for c in range(KD):
                        nc.tensor.matmul(
                            pp[:, :512],
                            wsb[:, c, jb * P:(jb + 1) * P],
                            xts[sch][:, c, :],
                            start=(c == 0),
                            stop=(c == KD - 1),
                        )
                    nc.vector.tensor_copy(
                        dst[:, jb, sch * 512:(sch + 1) * 512], pp[:])
                return f

            def vjob(sb):
                def f():
                    vp = psc.tile([P, J], F32, tag="sc", name=f"vp{sb}")
                    for c in range(KD):
                        nc.tensor.matmul(
                            vp[:, :J],
                            xv_t[sb // 4][:, c, (sb % 4) * P:(sb % 4 + 1) * P],
                            wv_sb[:, c, :],
                            start=(c == 0),
                            stop=(c == KD - 1),
                        )
                    nc.vector.tensor_copy(v_sb[:, sb, :, 0:DH], vp[:])
                return f

            def oproj_eb(qh, eb, evac="dve", tailpool=False):
                def f():
                    q0 = qh * 1024
                    ob = opool.tile([P, 1024], F16, tag="ob",
                                    name=f"ob{(qh * EB + eb) % 4}")
                    for stl in range(2):
                        s0 = q0 + stl * 512
                        pool = poacc if (tailpool and stl == 1) else psc
                        tg = "oacc" if (tailpool and stl == 1) else "sc"
                        po = pool.tile([P, 512], F32, tag=tg,
                                       name=f"po_{qh}_{eb}_{stl}")
                        for jbx in range(2):
                            nc.tensor.matmul(
                                po[:, :512],
                                wo_sb[:, jbx, eb * P:(eb + 1) * P],
                                ao_sb[:, jbx, s0:s0 + 512],
                                start=(jbx == 0),
                                stop=(jbx == 1),
                            )
                        d = ob[:, stl * 512:(stl + 1) * 512]
                        if evac == "act" or (evac == "mix" and stl == 0):
                            nc.scalar.copy(d, po[:, :512])
                        else:
                            nc.vector.tensor_copy(d, po[:, :512])
                    nc.sync.dma_start(out=out_t[eb][:, q0:q0 + 1024],
                                      in_=ob[:])
                return f

            # filler schedule: fillers[(task, kb)] run inside the kb loop
            # after scores/exp/attn@V; positions tuned against the model
            fillers = {}

            def put(i, kb, fn):
                fillers.setdefault((i, kb), []).append(fn)

            put(0, 3, proj_job(wk_sb, xk_t, kt_sb, 1, 0, "k"))
            put(0, 7, proj_job(wk_sb, xk_t, kt_sb, 2, 0, "k"))
            put(0, 10, proj_job(wk_sb, xk_t, kt_sb, 3, 0, "k"))
            put(0, 0, proj_job(wk_sb, xk_t, kt_sb, 0, 1, "k"))
            put(0, 1, proj_job(wq_sb, xq_t, qt_sb, 0, 1, "q"))
            put(0, 4, proj_job(wq_sb, xq_t, qt_sb, 1, 1, "q"))
            put(0, 5, proj_job(wk_sb, xk_t, kt_sb, 1, 1, "k"))
            put(0, 8, proj_job(wk_sb, xk_t, kt_sb, 2, 1, "k"))
            put(0, 12, proj_job(wk_sb, xk_t, kt_sb, 3, 1, "k"))
            for n in range(3):
                put(0, 13 + n, vjob(n))
            put(1, 0, vjob(3))
            for n in range(4):
                put(1, 1 + n, vjob(4 + n))
            for n in range(4):
                put(1, 5 + n, vjob(8 + n))
            for n in range(4):
                put(1, 9 + n, vjob(12 + n))
            put(2, 3, proj_job(wq_sb, xq_t, qt_sb, 2, 0, "q"))
            put(2, 11, proj_job(wq_sb, xq_t, qt_sb, 3, 0, "q"))
            put(3, 3, proj_job(wq_sb, xq_t, qt_sb, 2, 1, "q"))
            put(3, 11, proj_job(wq_sb, xq_t, qt_sb, 3, 1, "q"))
            # ao(qh0) is complete only after norm(3), which lands at the
            # END of task 4 (task-lag) — oproj(0) fillers start in task 5
            opos = [(5, 2), (5, 7), (5, 12), (6, 2), (6, 7), (6, 12),
                    (7, 2), (7, 7)]
            for eb in range(EB):
                put(*opos[eb], oproj_eb(0, eb))

            def normalize(h, q0, w, oacc, recip, nm):
                def f():
                    jbn = h // 2
                    off = DH * (h % 2)
                    bc = psc.tile([DH, w], F32, tag="sc", name=f"bc{nm}")
                    for n in range(w // 512):
                        nc.tensor.matmul(
                            bc[:, n * 512:(n + 1) * 512],
                            ones64[:],
                            recip[:, n * 512:(n + 1) * 512],
                            start=True,
                            stop=True,
                        )
                    bcast = npool.tile([DH, w], F16, tag="bcast",
                                       name=f"bst{nm}")
                    nc.vector.tensor_copy(bcast[:], bc[:])
                    nc.vector.tensor_mul(
                        ao_sb[off:off + DH, jbn, q0:q0 + w],
                        oacc[0:DH, :],
                        bcast[:],
                    )
                return f

            # pre-task projections: first scores need Q sch0-1 + K sch0 (jb0)
            proj_job(wk_sb, xk_t, kt_sb, 0, 0, "k")()
            proj_job(wq_sb, xq_t, qt_sb, 0, 0, "q")()
            proj_job(wq_sb, xq_t, qt_sb, 1, 0, "q")()

            # attn@V(i) runs one full task later (task-lag): it reads only
            # buffered exps + V, so the PE never stalls on the live exp
            # stream (a per-kb dependency stall would reset the clock ramp
            # and lock the PE at 1.2GHz). The last task additionally
            # consumes its own exps lag-1 and compresses attn@V(6) into
            # its first half so the tail is just norm(7) + the qh1 oproj.
            tasks = [(qh, hh) for qh in range(QH) for hh in range(HL)]
            NT = len(tasks)

            def attn_v(oacc, hh, ex, kb):
                for n in range(2):
                    nc.tensor.matmul(
                        oacc[:, n * 512:(n + 1) * 512],
                        v_sb[:, kb, hh, :],
                        ex[kb][:, n * 512:(n + 1) * 512],
                        start=(kb == 0),
                        stop=(kb == KB - 1),
                    )

            prev = None  # (qh, h, ex) whose attn@V runs this iter
            exn = 0
            for i in range(NT):
                qh, h = tasks[i]
                q0 = qh * 1024
                jb = h // 2
                off = DH * (h % 2)
                cur = (qh, h, {})
                oacc = poacc.tile([DH + 1, 1024], F32, tag="oacc",
                                  name=f"oacc{i % 2}") if prev else None
                oacc7 = poacc.tile([DH + 1, 1024], F32, tag="oacc",
                                   name="oacc7") if i == NT - 1 else None
                for kb in range(KB):
                    sc = psc.tile([P, 1024], F32, tag="sc", name=f"sc{kb % 2}")
                    for n in range(2):
                        nc.tensor.matmul(
                            sc[:, n * 512:(n + 1) * 512],
                            kt_sb[off:off + DH, jb, kb * P:(kb + 1) * P],
                            qt_sb[off:off + DH, jb,
                                  q0 + n * 512:q0 + (n + 1) * 512],
                            start=True,
                            stop=True,
                        )
                    cur[2][kb] = expp.tile([P, 1024], F16, tag="ex",
                                           name=f"ex{exn % 21}")
                    exn += 1
                    nc.scalar.activation(cur[2][kb][:], sc[:], AF.Exp)
                    if prev is not None:
                        if oacc7 is not None:
                            if kb < KB // 2:
                                attn_v(oacc, prev[1], prev[2], 2 * kb)
                                attn_v(oacc, prev[1], prev[2], 2 * kb + 1)
                        else:
                            attn_v(oacc, prev[1], prev[2], kb)
                    if oacc7 is not None and kb >= 1:
                        attn_v(oacc7, h, cur[2], kb - 1)
                    if oacc7 is not None and kb == KB // 2:
                        recip6 = npool.tile([1, 1024], F32R, tag="recip",
                                            name="recip6")
                        with nc.allow_low_precision(reason="softmax denom"):
                            nc.vector.reciprocal(recip6[:],
                                                 oacc[DH:DH + 1, :])
                    if oacc7 is not None and kb == KB // 2 + 2:
                        normalize(prev[1], prev[0] * 1024, 1024, oacc,
                                  recip6, "n6")()
                    for fn in fillers.pop((i, kb), ()):
                        fn()
                if prev is not None and oacc7 is None:
                    recip = npool.tile([1, 1024], F32R, tag="recip",
                                       name=f"recip{i % 2}")
                    with nc.allow_low_precision(reason="fp32r softmax denom"):
                        nc.vector.reciprocal(recip[:], oacc[DH:DH + 1, :])
                    normalize(prev[1], prev[0] * 1024, 1024, oacc, recip,
                              f"n{i % 2}")()
                if oacc7 is not None:
                    attn_v(oacc7, h, cur[2], KB - 1)
                    recip7 = npool.tile([1, 1024], F32R, tag="recip",
                                        name="recip7")
                    with nc.allow_low_precision(reason="fp32r softmax denom"):
                        nc.vector.reciprocal(recip7[:], oacc7[DH:DH + 1, :])
                    normalize(h, q0, 1024, oacc7, recip7, "nd")()
                    prev = None
                else:
                    prev = cur

            # tail: qh1 output projection, po tiles spread over both pools
            for eb in range(EB):
                oproj_eb(1, eb, evac="mix", tailpool=True)()

    nc.finalize()
    return nc


_NC_CACHE = None


def _get_nc():
    global _NC_CACHE
    if _NC_CACHE is None:
        _NC_CACHE = build_nc()
    return _NC_CACHE


def make_in_maps(query, key, value, Wq, Wk, Wv, Wo):
    """Build the 8 per-core input dicts from the full tensors."""
    query = np.asarray(query, np.float32)
    key = np.asarray(key, np.float32)
    value = np.asarray(value, np.float32)
    Wq = np.asarray(Wq, np.float32)
    Wk = np.asarray(Wk, np.float32)
    Wv = np.asarray(Wv, np.float32)
    Wo = np.asarray(Wo, np.float32)

    def pmajor(a2d, inner):  # [Drows, inner] -> [P, Drows//P, inner]
        return np.ascontiguousarray(
            a2d.reshape(KD, P, inner).transpose(1, 0, 2)
        )

    def schunk(a2d):  # X^T [D, S] -> [P, NS, KD, 512]
        return np.ascontiguousarray(
            a2d.reshape(KD, P, NS, 512).transpose(1, 2, 0, 3)
        )

    scale = np.float32(1.0 / np.sqrt(DH))
    xs = {}
    for b in range(B):
        xs[b] = {
            "xq": schunk(np.ascontiguousarray(query[b].T)).astype(np.float16),
            "xk": schunk(np.ascontiguousarray(key[b].T)).astype(np.float16),
            "xv": schunk(np.ascontiguousarray(value[b].T)).astype(np.float16),
        }
    ws = {}
    for hg in range(4):
        sl = slice(hg * J, (hg + 1) * J)
        wo_t = np.ascontiguousarray(Wo[:, sl].T)  # [256, 1024]
        ws[hg] = {
            "wq": pmajor(np.ascontiguousarray(Wq[sl].T * scale), J).astype(
                np.float16),
            "wk": pmajor(np.ascontiguousarray(Wk[sl].T), J).astype(np.float16),
            "wv": pmajor(np.ascontiguousarray(Wv[sl].T), J).astype(np.float16),
            "wo": np.ascontiguousarray(
                wo_t.reshape(2, P, D).transpose(1, 0, 2)
            ).astype(np.float16),
        }
    in_maps = []
    for c in range(NCORES):
        b, hg = c // 4, c % 4
        m = {}
        m.update(xs[b])
        m.update(ws[hg])
        in_maps.append(m)
    return in_maps


def assemble(results, bo):
    """Sum the 4 per-core partials per batch, add bo."""
    bo = np.asarray(bo, np.float32)
    out = np.zeros((B, S, D), np.float32)
    for c in range(NCORES):
        b = c // 4
        part = results[c]["out_t"].astype(np.float32).reshape(D, S).T
        out[b] += part
    out += bo[None, None, :]
    return out


def kernel(query, key, value, Wq, Wk, Wv, Wo, bo):
    import os
    import time

    os.environ.setdefault("NEURON_RT_RESET_CORES", "1")
    from concourse.bass_utils import run_bass_kernel_spmd

    nc = _get_nc()
    in_maps = make_in_maps(query, key, value, Wq, Wk, Wv, Wo)
    last_exc = None
    for attempt in range(3):
        try:
            res = run_bass_kernel_spmd(nc, in_maps, list(range(NCORES)))
            return assemble(res.results, bo)
        except Exception as e:  # transient NRT_EXEC_UNIT_UNRECOVERABLE etc.
            last_exc = e
            time.sleep(2.0)
    raise last_exc
